# revision 2
# baseline (speedup 1.0000x reference)
"""NodeAttention GNN message passing kernel for 8 trn2 NeuronCores — v2.

Hybrid-engine dataflow (per core, one batch element; N=512, D=768, DEP=64):

  s_dep[i,j] = adj[i,j,:] @ w_dep          (the only full-volume compute)
  em[i,j]    = exp(s_dep) * (s_dep != 0)   (s_asp cancels in softmax)
  agg[i,:]   = (sum_j em * e^{s_nbr_j} * [feat_j|1]) ; denominator = ones col
  out[i]     = update[i] ? agg/den : feat[i]

The e^{s_nbr_j} factor is folded into the features operand (featq) as a
per-partition scale, so neither route needs a free-dim bias for the softmax.

Route A (i-blocks 0-1, PE path): PE-transpose [128i, 2j*64k] blocks into
4-pair PSUM stages, one ACT copy per stage, then per pair a matmul with a
shifted two-column weight stationary accumulating s_dep^T[j', i] in PSUM.

Route B (i-blocks 2-3, DVE path): in-place broadcast multiply by w_dep and
segmented tensor_reduce in the natural [i, (j,k)] layout; the small
[512 x 512] score matrix is then PE-transposed into the same PSUM tiles.

Both routes meet in sd_full[jb] PSUM [128 j', 512 i]; per-j-block epilogue
(exp + mask) runs inside the stream; agg matmuls (f32r, 1 cyc/row) and the
blend form a short tail.
"""
import sys

if "/opt/trn_rl_repo" not in sys.path:
    sys.path.insert(0, "/opt/trn_rl_repo")

import numpy as np
from contextlib import ExitStack

import concourse.bass as bass
from concourse import bacc
import concourse.mybir as mybir
import concourse.tile as tile
from concourse.bass_utils import run_bass_kernel_spmd

F32 = mybir.dt.float32
F32R = mybir.dt.float32r
BF16 = mybir.dt.bfloat16

N = 512     # nodes
D = 768     # feature dim
DEP = 64    # edge embedding dim
P = 128     # partitions
NB = N // P         # 4 node blocks
JC = 32             # j's per adj DMA tile
NJC = N // JC       # 16 j-chunks
PPC = JC // 2       # 16 j-pairs per chunk
CPJ = P // JC       # 4 chunks per j-block
GP = 4              # j-pairs per PSUM stage
NA = 2 * P          # i's handled by route A (blocks 0-1)

_CACHED = {}


def _build():
    nc = bacc.Bacc()
    adj = nc.dram_tensor("adj", [N, N * DEP], F32, kind="ExternalInput")
    feat = nc.dram_tensor("feat", [N, D], F32, kind="ExternalInput")
    aspf = nc.dram_tensor("aspf", [N], F32, kind="ExternalInput")
    ident = nc.dram_tensor("ident", [P, P], F32R, kind="ExternalInput")
    wpad = nc.dram_tensor("wpad", [P, 126 + P], F32R, kind="ExternalInput")
    wnbr = nc.dram_tensor("wnbr", [D], F32, kind="ExternalInput")
    wdeprep = nc.dram_tensor("wdeprep", [JC * DEP], BF16, kind="ExternalInput")
    out = nc.dram_tensor("out", [N, D], F32, kind="ExternalOutput")

    with ExitStack() as ctx:
        tc = ctx.enter_context(tile.TileContext(nc))
        const = ctx.enter_context(tc.tile_pool(name="const", bufs=1))
        tpool = ctx.enter_context(tc.tile_pool(name="tpool", bufs=4))
        spool = ctx.enter_context(tc.tile_pool(name="spool", bufs=3))
        epool = ctx.enter_context(tc.tile_pool(name="epool", bufs=1))
        opool = ctx.enter_context(tc.tile_pool(name="opool", bufs=2))
        sd_ps = ctx.enter_context(tc.tile_pool(name="sd_ps", bufs=1, space="PSUM"))

        def rep_dma(dst, src_ap, n_part):
            nc.scalar.dma_start(
                dst,
                bass.AP(tensor=src_ap.tensor, offset=src_ap.offset,
                        ap=[[0, n_part]] + list(src_ap.ap)),
            )

        # ---- constants / small inputs (scalar-engine DMA queue so the
        # sync-engine queue is free for the adj stream) ----
        ident_sb = const.tile([P, P], F32R)
        nc.scalar.dma_start(ident_sb[:], ident[:, :])
        identf_sb = const.tile([P, P], F32)
        nc.scalar.dma_start(identf_sb[:], ident[:, :].bitcast(F32))
        wpad_sb = const.tile([P, 126 + P], F32R)
        nc.scalar.dma_start(wpad_sb[:], wpad[:, :])
        wnbr_sb = const.tile([P, D], F32)
        rep_dma(wnbr_sb[:], wnbr[:], P)
        wdep_sb = const.tile([P, JC * DEP], BF16)
        rep_dma(wdep_sb[:], wdeprep[:], P)

        # original features (for final blend) and scaled features featq
        featp = []
        for b in range(NB):
            f = const.tile([P, D], F32, tag=f"featp{b}", name=f"featp{b}")
            nc.scalar.dma_start(f[:], feat[b * P:(b + 1) * P, :])
            featp.append(f)

        aspf_sb = const.tile([P, NB], F32)
        for b in range(NB):
            nc.scalar.dma_start(aspf_sb[:, b:b + 1], aspf[b * P:(b + 1) * P])

        # s_nbr[j] per node block, then featq[j,:] = exp(s_nbr[j])*[feat|1]
        snbr_sb = const.tile([P, NB], F32)
        esnbr_sb = const.tile([P, NB], F32)
        for b in range(NB):
            fw = opool.tile([P, D], F32, tag="fw")
            nc.vector.tensor_mul(fw[:], featp[b][:], wnbr_sb[:])
            nc.vector.tensor_reduce(
                snbr_sb[:, b:b + 1], fw[:],
                axis=mybir.AxisListType.X, op=mybir.AluOpType.add,
            )
        nc.scalar.activation(
            esnbr_sb[:], snbr_sb[:], mybir.ActivationFunctionType.Exp,
        )
        featq = []
        for b in range(NB):
            q = const.tile([P, D + 1], BF16, tag=f"featq{b}", name=f"featq{b}")
            nc.scalar.mul(q[:, 0:D], featp[b][:], esnbr_sb[:, b:b + 1])
            nc.scalar.copy(q[:, D:D + 1], esnbr_sb[:, b:b + 1])
            featq.append(q)

        # sd_full[jb][j', i]: cols 0:256 from route A, 256:512 from route B
        sd = [sd_ps.tile([P, N], F32, tag=f"sd{jb}", name=f"sd{jb}")
              for jb in range(NB)]
        # em_T[jb][j', i] = exp(sd) * (sd != 0)
        em = [epool.tile([P, N], BF16, tag=f"e{jb}", name=f"e{jb}")
              for jb in range(NB)]
        # route B scores in i-domain: [128 i, 512 j] per i-block 2,3
        sdep_b = [epool.tile([P, N], F32, tag=f"sb{b}", name=f"sb{b}")
                  for b in range(2)]

        adj_v = adj.rearrange("(nb p) (jc f) -> nb p jc f", p=P, f=JC * DEP)

        with tc.tile_pool(name="s_ps", bufs=2, space="PSUM") as s_ps:
            # adj DMA queues per i-block: 0,1 sync (f32r) / 2,3 gpsimd
            # (SWDGE casting DMA delivers route-B tiles as bf16 so the DVE
            # multiply runs in its 2x perf mode)
            for jc in range(NJC):
                jb = jc // CPJ
                tiles = []
                for b in (0, 1):
                    t = tpool.tile([P, JC * DEP], F32R, tag=f"t{b}", name=f"t{b}")
                    nc.sync.dma_start(t[:], adj_v[b, :, jc, :].bitcast(F32R))
                    tiles.append(t)
                for b in (2, 3):
                    t = tpool.tile([P, JC * DEP], BF16, tag=f"t{b}", name=f"t{b}")
                    nc.gpsimd.dma_start(t[:], adj_v[b, :, jc, :])
                    tiles.append(t)

                # ---- route B (DVE): i-blocks 2,3 ----
                # bf16 in-place halving-tree keeps everything in the DVE 2x
                # perf mode; only the final 8-wide reduce runs at 1x (f32 out)
                for b in (2, 3):
                    nc.vector.tensor_mul(tiles[b][:], tiles[b][:], wdep_sb[:])
                    tv = tiles[b][:].rearrange("p (j k) -> p j k", k=DEP)
                    for h in (32, 16, 8):
                        nc.vector.tensor_add(
                            tv[:, :, 0:h], tv[:, :, 0:h], tv[:, :, h:2 * h],
                        )
                    nc.vector.tensor_reduce(
                        sdep_b[b - 2][:, jc * JC:(jc + 1) * JC],
                        tv[:, :, 0:8],
                        axis=mybir.AxisListType.X, op=mybir.AluOpType.add,
                    )

                # ---- route A (PE + ACT): i-blocks 0,1 ----
                # matmuls trail their stage by one group so the ACT copy has
                # a full group of PE transposes to hide behind
                s_sbs = [None] * (PPC // GP)

                def scatter(g):
                    for q in range(GP):
                        mm = (jc % CPJ) * PPC + g * GP + q
                        nc.tensor.matmul(
                            sd[jb][:, 0:NA],
                            wpad_sb[:, 126 - 2 * mm:126 - 2 * mm + P],
                            s_sbs[g][:, q * NA:(q + 1) * NA],
                            start=(mm == 0), stop=(mm == PPC * CPJ - 1),
                        )

                for g in range(PPC // GP):
                    stage = s_ps.tile([P, GP * NA], F32R, tag="stage")
                    for q in range(GP):
                        tp = g * GP + q
                        for b in (0, 1):
                            nc.tensor.transpose(
                                stage[:, q * NA + b * P: q * NA + (b + 1) * P],
                                tiles[b][:, 2 * tp * DEP:(2 * tp + 2) * DEP],
                                ident_sb[:],
                            )
                    s_sb = spool.tile([P, GP * NA], F32R, tag="s_sb")
                    nc.scalar.copy(s_sb[:], stage[:])
                    s_sbs[g] = s_sb
                    if g >= 1:
                        scatter(g - 1)
                scatter(PPC // GP - 1)

                # ---- per-j-block epilogue (exp + mask into em_T) ----
                if jc % CPJ == CPJ - 1:
                    for b in (2, 3):
                        nc.tensor.transpose(
                            sd[jb][:, b * P:(b + 1) * P],
                            sdep_b[b - 2][:, jb * P:(jb + 1) * P],
                            identf_sb[:],
                        )
                    nc.scalar.activation(
                        em[jb][:], sd[jb][:], mybir.ActivationFunctionType.Exp,
                    )
                    m01 = opool.tile([P, N], BF16, tag="m01")
                    nc.vector.tensor_scalar(
                        m01[:], sd[jb][:], 0.0, None,
                        op0=mybir.AluOpType.not_equal,
                    )
                    mul_eng = nc.vector if jb == NB - 1 else nc.gpsimd
                    mul_eng.tensor_mul(em[jb][:], em[jb][:], m01[:])

        # ---- tail: agg matmuls + blend ----
        with tc.tile_pool(name="agg_ps", bufs=2, space="PSUM") as agg_ps:
            for ib in range(NB):
                agg = agg_ps.tile([P, D + 1], F32, tag="agg")
                for jb in range(NB):
                    for c0, c1 in ((0, 512), (512, D + 1)):
                        nc.tensor.matmul(
                            agg[:, c0:c1],
                            em[jb][:, ib * P:(ib + 1) * P],
                            featq[jb][:, c0:c1],
                            start=(jb == 0),
                            stop=(jb == NB - 1),
                        )
                den = opool.tile([P, 4], F32, tag="den")
                nc.vector.tensor_scalar(
                    den[:, 0:1], agg[:, D:D + 1], 1e-30, None,
                    op0=mybir.AluOpType.max,
                )
                nc.vector.tensor_scalar(
                    den[:, 1:2], agg[:, D:D + 1], 0.0, None,
                    op0=mybir.AluOpType.is_gt,
                )
                nc.vector.reciprocal(den[:, 2:3], den[:, 0:1])
                u = opool.tile([P, 3], F32, tag="u")
                nc.vector.tensor_mul(u[:, 0:1], den[:, 1:2], aspf_sb[:, ib:ib + 1])
                nc.vector.tensor_mul(u[:, 1:2], u[:, 0:1], den[:, 2:3])
                nc.vector.tensor_scalar(
                    u[:, 2:3], u[:, 0:1], -1.0, 1.0,
                    op0=mybir.AluOpType.mult, op1=mybir.AluOpType.add,
                )
                o1 = opool.tile([P, D], F32, tag="o1")
                nc.scalar.mul(o1[:], agg[:, 0:D], u[:, 1:2])
                o2 = opool.tile([P, D], F32, tag="o2")
                nc.vector.tensor_scalar_mul(o2[:], featp[ib][:], u[:, 2:3])
                nc.vector.tensor_add(o1[:], o1[:], o2[:])
                out_q = nc.sync if ib % 2 == 0 else nc.scalar
                out_q.dma_start(out[ib * P:(ib + 1) * P, :], o1[:])

    nc.finalize()
    return nc


def _get_nc():
    if "nc" not in _CACHED:
        _CACHED["nc"] = _build()
    return _CACHED["nc"]


def kernel(features, aspect_onehot, adj_matrix, w_att):
    features = np.ascontiguousarray(features, dtype=np.float32)
    adj_matrix = np.ascontiguousarray(adj_matrix, dtype=np.float32)
    w_att = np.asarray(w_att, dtype=np.float32)
    B = features.shape[0]

    import ml_dtypes

    w_dep = w_att[D:D + DEP]
    wpad = np.zeros((P, 126 + P), dtype=np.float32)
    wpad[0:DEP, 126] = w_dep
    wpad[DEP:2 * DEP, 127] = w_dep
    ident = np.eye(P, dtype=np.float32)
    aspf = aspect_onehot.astype(np.float32)
    wdeprep = np.tile(w_dep, JC).astype(ml_dtypes.bfloat16)

    nc = _get_nc()
    in_maps = [
        {
            "adj": adj_matrix[b].reshape(N, N * DEP),
            "feat": features[b],
            "aspf": aspf[b],
            "ident": ident,
            "wpad": wpad,
            "wnbr": w_att[0:D].copy(),
            "wdeprep": wdeprep,
        }
        for b in range(B)
    ]
    res = run_bass_kernel_spmd(nc, in_maps, list(range(B)))
    return np.stack([res.results[b]["out"] for b in range(B)], axis=0)


# revision 3
# speedup vs baseline: 1.0960x; 1.0960x over previous
"""NodeAttention GNN message passing kernel for 8 trn2 NeuronCores — v2.

Hybrid-engine dataflow (per core, one batch element; N=512, D=768, DEP=64):

  s_dep[i,j] = adj[i,j,:] @ w_dep          (the only full-volume compute)
  em[i,j]    = exp(s_dep) * (s_dep != 0)   (s_asp cancels in softmax)
  agg[i,:]   = (sum_j em * e^{s_nbr_j} * [feat_j|1]) ; denominator = ones col
  out[i]     = update[i] ? agg/den : feat[i]

The e^{s_nbr_j} factor is folded into the features operand (featq) as a
per-partition scale, so neither route needs a free-dim bias for the softmax.

Route A (i-blocks 0-1, PE path): PE-transpose [128i, 2j*64k] blocks into
4-pair PSUM stages, one ACT copy per stage, then per pair a matmul with a
shifted two-column weight stationary accumulating s_dep^T[j', i] in PSUM.

Route B (i-blocks 2-3, DVE path): in-place broadcast multiply by w_dep and
segmented tensor_reduce in the natural [i, (j,k)] layout; the small
[512 x 512] score matrix is then PE-transposed into the same PSUM tiles.

Both routes meet in sd_full[jb] PSUM [128 j', 512 i]; per-j-block epilogue
(exp + mask) runs inside the stream; agg matmuls (f32r, 1 cyc/row) and the
blend form a short tail.
"""
import sys

if "/opt/trn_rl_repo" not in sys.path:
    sys.path.insert(0, "/opt/trn_rl_repo")

import numpy as np
from contextlib import ExitStack

import concourse.bass as bass
from concourse import bacc
import concourse.mybir as mybir
import concourse.tile as tile
from concourse.bass_utils import run_bass_kernel_spmd

F32 = mybir.dt.float32
F32R = mybir.dt.float32r
BF16 = mybir.dt.bfloat16

N = 512     # nodes
D = 768     # feature dim
DEP = 64    # edge embedding dim
P = 128     # partitions
NB = N // P         # 4 node blocks
JC = 32             # j's per adj DMA tile
NJC = N // JC       # 16 j-chunks
PPC = JC // 2       # 16 j-pairs per chunk
CPJ = P // JC       # 4 chunks per j-block
GP = 4              # j-pairs per PSUM stage
NA = P              # i's handled by route A (block 0); DVE takes blocks 1-3

_CACHED = {}


def _build():
    nc = bacc.Bacc()
    adj = nc.dram_tensor("adj", [N, N * DEP], F32, kind="ExternalInput")
    feat = nc.dram_tensor("feat", [N, D], F32, kind="ExternalInput")
    aspf = nc.dram_tensor("aspf", [N], F32, kind="ExternalInput")
    ident = nc.dram_tensor("ident", [P, P], F32R, kind="ExternalInput")
    wpad = nc.dram_tensor("wpad", [P, 126 + P], BF16, kind="ExternalInput")
    wnbr = nc.dram_tensor("wnbr", [D], F32, kind="ExternalInput")
    wdeprep = nc.dram_tensor("wdeprep", [JC * DEP], BF16, kind="ExternalInput")
    out = nc.dram_tensor("out", [N, D], F32, kind="ExternalOutput")

    with ExitStack() as ctx:
        tc = ctx.enter_context(tile.TileContext(nc))
        const = ctx.enter_context(tc.tile_pool(name="const", bufs=1))
        tpool = ctx.enter_context(tc.tile_pool(name="tpool", bufs=4))
        spool = ctx.enter_context(tc.tile_pool(name="spool", bufs=3))
        epool = ctx.enter_context(tc.tile_pool(name="epool", bufs=1))
        opool = ctx.enter_context(tc.tile_pool(name="opool", bufs=2))
        sd_ps = ctx.enter_context(tc.tile_pool(name="sd_ps", bufs=1, space="PSUM"))

        def rep_dma(dst, src_ap, n_part):
            nc.scalar.dma_start(
                dst,
                bass.AP(tensor=src_ap.tensor, offset=src_ap.offset,
                        ap=[[0, n_part]] + list(src_ap.ap)),
            )

        # ---- constants / small inputs (scalar-engine DMA queue so the
        # sync-engine queue is free for the adj stream) ----
        ident_sb = const.tile([P, P], F32R)
        nc.scalar.dma_start(ident_sb[:], ident[:, :])
        identf_sb = const.tile([P, P], F32)
        nc.scalar.dma_start(identf_sb[:], ident[:, :].bitcast(F32))
        wpad_sb = const.tile([P, 126 + P], BF16)
        nc.scalar.dma_start(wpad_sb[:], wpad[:, :])
        wnbr_sb = const.tile([P, D], F32)
        rep_dma(wnbr_sb[:], wnbr[:], P)
        wdep_sb = const.tile([P, JC * DEP], BF16)
        rep_dma(wdep_sb[:], wdeprep[:], P)

        # original features (for final blend) and scaled features featq
        featp = []
        for b in range(NB):
            f = const.tile([P, D], F32, tag=f"featp{b}", name=f"featp{b}")
            nc.scalar.dma_start(f[:], feat[b * P:(b + 1) * P, :])
            featp.append(f)

        aspf_sb = const.tile([P, NB], F32)
        for b in range(NB):
            nc.scalar.dma_start(aspf_sb[:, b:b + 1], aspf[b * P:(b + 1) * P])

        # s_nbr[j] per node block, then featq[j,:] = exp(s_nbr[j])*[feat|1]
        snbr_sb = const.tile([P, NB], F32)
        esnbr_sb = const.tile([P, NB], F32)
        for b in range(NB):
            fw = opool.tile([P, D], F32, tag="fw")
            nc.vector.tensor_mul(fw[:], featp[b][:], wnbr_sb[:])
            nc.vector.tensor_reduce(
                snbr_sb[:, b:b + 1], fw[:],
                axis=mybir.AxisListType.X, op=mybir.AluOpType.add,
            )
        nc.scalar.activation(
            esnbr_sb[:], snbr_sb[:], mybir.ActivationFunctionType.Exp,
        )
        featq = []
        for b in range(NB):
            q = const.tile([P, D + 1], BF16, tag=f"featq{b}", name=f"featq{b}")
            nc.scalar.mul(q[:, 0:D], featp[b][:], esnbr_sb[:, b:b + 1])
            nc.scalar.copy(q[:, D:D + 1], esnbr_sb[:, b:b + 1])
            featq.append(q)

        # sd_full[jb][j', i]: cols 0:256 from route A, 256:512 from route B
        sd = [sd_ps.tile([P, N], F32, tag=f"sd{jb}", name=f"sd{jb}")
              for jb in range(NB)]
        # em_T[jb][j', i] = exp(sd) * (sd != 0)
        em = [epool.tile([P, N], BF16, tag=f"e{jb}", name=f"e{jb}")
              for jb in range(NB)]
        # route B scores in i-domain: [128 i, 512 j] per i-block 1,2,3
        sdep_b = [epool.tile([P, N], F32, tag=f"sb{b}", name=f"sb{b}")
                  for b in range(3)]

        adj_v = adj.rearrange("(nb p) (jc f) -> nb p jc f", p=P, f=JC * DEP)

        with tc.tile_pool(name="s_ps", bufs=2, space="PSUM") as s_ps:
            # adj DMA queues per i-block: 0 sync (f32r) / 1-3 gpsimd
            # (SWDGE casting DMA delivers route-B tiles as bf16 so the DVE
            # multiply runs in its 2x perf mode)
            for jc in range(NJC):
                jb = jc // CPJ
                tiles = []
                t = tpool.tile([P, JC * DEP], F32R, tag="t0", name="t0")
                nc.sync.dma_start(t[:], adj_v[0, :, jc, :].bitcast(F32R))
                tiles.append(t)
                for b in (1, 2, 3):
                    t = tpool.tile([P, JC * DEP], BF16, tag=f"t{b}", name=f"t{b}")
                    nc.gpsimd.dma_start(t[:], adj_v[b, :, jc, :])
                    tiles.append(t)

                # ---- route B (DVE): i-blocks 1-3 ----
                # bf16 in-place halving-tree keeps everything in the DVE 2x
                # perf mode; only the final 8-wide reduce runs at 1x (f32 out)
                for b in (1, 2, 3):
                    nc.vector.tensor_mul(tiles[b][:], tiles[b][:], wdep_sb[:])
                    tv = tiles[b][:].rearrange("p (j k) -> p j k", k=DEP)
                    for h in (32, 16, 8):
                        nc.vector.tensor_add(
                            tv[:, :, 0:h], tv[:, :, 0:h], tv[:, :, h:2 * h],
                        )
                    nc.vector.tensor_reduce(
                        sdep_b[b - 1][:, jc * JC:(jc + 1) * JC],
                        tv[:, :, 0:8],
                        axis=mybir.AxisListType.X, op=mybir.AluOpType.add,
                    )

                # ---- route A (PE + ACT): i-block 0 ----
                # matmuls trail their stage by one group so the ACT copy has
                # a full group of PE transposes to hide behind
                s_sbs = [None] * (PPC // GP)

                def scatter(g):
                    for q in range(GP):
                        mm = (jc % CPJ) * PPC + g * GP + q
                        nc.tensor.matmul(
                            sd[jb][:, 0:NA],
                            wpad_sb[:, 126 - 2 * mm:126 - 2 * mm + P],
                            s_sbs[g][:, q * NA:(q + 1) * NA],
                            start=(mm == 0), stop=(mm == PPC * CPJ - 1),
                        )

                for g in range(PPC // GP):
                    stage = s_ps.tile([P, GP * NA], F32R, tag="stage")
                    for q in range(GP):
                        tp = g * GP + q
                        nc.tensor.transpose(
                            stage[:, q * NA:(q + 1) * NA],
                            tiles[0][:, 2 * tp * DEP:(2 * tp + 2) * DEP],
                            ident_sb[:],
                        )
                    s_sb = spool.tile([P, GP * NA], BF16, tag="s_sb")
                    nc.scalar.copy(s_sb[:], stage[:])
                    s_sbs[g] = s_sb
                    if g >= 1:
                        scatter(g - 1)
                scatter(PPC // GP - 1)

                # ---- per-j-block epilogue (exp + mask into em_T) ----
                if jc % CPJ == CPJ - 1:
                    for b in (1, 2, 3):
                        nc.tensor.transpose(
                            sd[jb][:, b * P:(b + 1) * P],
                            sdep_b[b - 1][:, jb * P:(jb + 1) * P],
                            identf_sb[:],
                        )
                    nc.scalar.activation(
                        em[jb][:], sd[jb][:], mybir.ActivationFunctionType.Exp,
                    )
                    m01 = opool.tile([P, N], BF16, tag="m01")
                    nc.vector.tensor_scalar(
                        m01[:], sd[jb][:], 0.0, None,
                        op0=mybir.AluOpType.not_equal,
                    )
                    nc.vector.tensor_mul(em[jb][:], em[jb][:], m01[:])

        # ---- tail: agg matmuls + blend ----
        with tc.tile_pool(name="agg_ps", bufs=2, space="PSUM") as agg_ps:
            for ib in range(NB):
                agg = agg_ps.tile([P, D + 1], F32, tag="agg")
                for jb in range(NB):
                    for c0, c1 in ((0, 512), (512, D + 1)):
                        nc.tensor.matmul(
                            agg[:, c0:c1],
                            em[jb][:, ib * P:(ib + 1) * P],
                            featq[jb][:, c0:c1],
                            start=(jb == 0),
                            stop=(jb == NB - 1),
                        )
                den = opool.tile([P, 4], F32, tag="den")
                nc.vector.tensor_scalar(
                    den[:, 0:1], agg[:, D:D + 1], 1e-30, None,
                    op0=mybir.AluOpType.max,
                )
                nc.vector.tensor_scalar(
                    den[:, 1:2], agg[:, D:D + 1], 0.0, None,
                    op0=mybir.AluOpType.is_gt,
                )
                nc.vector.reciprocal(den[:, 2:3], den[:, 0:1])
                u = opool.tile([P, 3], F32, tag="u")
                nc.vector.tensor_mul(u[:, 0:1], den[:, 1:2], aspf_sb[:, ib:ib + 1])
                nc.vector.tensor_mul(u[:, 1:2], u[:, 0:1], den[:, 2:3])
                nc.vector.tensor_scalar(
                    u[:, 2:3], u[:, 0:1], -1.0, 1.0,
                    op0=mybir.AluOpType.mult, op1=mybir.AluOpType.add,
                )
                o1 = opool.tile([P, D], F32, tag="o1")
                nc.scalar.mul(o1[:], agg[:, 0:D], u[:, 1:2])
                o2 = opool.tile([P, D], F32, tag="o2")
                nc.vector.tensor_scalar_mul(o2[:], featp[ib][:], u[:, 2:3])
                nc.vector.tensor_add(o1[:], o1[:], o2[:])
                out_q = nc.sync if ib % 2 == 0 else nc.scalar
                out_q.dma_start(out[ib * P:(ib + 1) * P, :], o1[:])

    nc.finalize()
    return nc


def _get_nc():
    if "nc" not in _CACHED:
        _CACHED["nc"] = _build()
    return _CACHED["nc"]


def kernel(features, aspect_onehot, adj_matrix, w_att):
    features = np.ascontiguousarray(features, dtype=np.float32)
    adj_matrix = np.ascontiguousarray(adj_matrix, dtype=np.float32)
    w_att = np.asarray(w_att, dtype=np.float32)
    B = features.shape[0]

    import ml_dtypes

    w_dep = w_att[D:D + DEP]
    wpad = np.zeros((P, 126 + P), dtype=np.float32)
    wpad[0:DEP, 126] = w_dep
    wpad[DEP:2 * DEP, 127] = w_dep
    wpad = wpad.astype(ml_dtypes.bfloat16)
    ident = np.eye(P, dtype=np.float32)
    aspf = aspect_onehot.astype(np.float32)
    wdeprep = np.tile(w_dep, JC).astype(ml_dtypes.bfloat16)

    nc = _get_nc()
    in_maps = [
        {
            "adj": adj_matrix[b].reshape(N, N * DEP),
            "feat": features[b],
            "aspf": aspf[b],
            "ident": ident,
            "wpad": wpad,
            "wnbr": w_att[0:D].copy(),
            "wdeprep": wdeprep,
        }
        for b in range(B)
    ]
    res = run_bass_kernel_spmd(nc, in_maps, list(range(B)))
    return np.stack([res.results[b]["out"] for b in range(B)], axis=0)
